# revision 25
# baseline (speedup 1.0000x reference)
"""Trainium2 Bass kernel for OESM CrossEntropy (two-stage top-k band mean).

reference semantics:
    loss[i] = -log_softmax(x)[i, target[i]]            # [B]
    keep the k1 = int(0.9*B) smallest losses, then the k2 = int(0.7*k1)
    largest of those, return their mean.
Equivalently: mean of the losses with ascending rank in [k1-k2, k1).

Strategy (8 NeuronCores, SPMD):
  - rows sharded 512/core; per row: sum(exp(x)) via ScalarE Exp with
    accum_out, x[i, target[i]] via indirect DMA gather (hoisted to t=0),
    g = s * exp(-x_t) = exp(loss), a strictly monotone transform of loss.
  - the [B] g-vector is exchanged with direct SBUF->SBUF remote DMA
    broadcasts (XOR-relative dests), one per row tile, as soon as each
    tile's g column is ready -- tiles 0..2 land while tile 3 still
    streams, so their rank work fully overlaps the stream. The CC
    AllGather software path (~13-32us latency) is bypassed entirely;
    a dummy 4-byte CC AllGather at t=0 keeps comm init alive.
  - each core ranks its own 512 values against all 4096 (DVE
    tensor_scalar is_lt with accum_out over a PSUM ones-matmul
    broadcast of the gathered values). The remote gather order is a
    per-receiver XOR permutation, which is harmless: rank counts are
    order-invariant.
  - band sum with strict ranks (no tie correction; exact for distinct
    values): S(k) = sum(v, rank<k). Per-core partials are
    partition-reduced (gpsimd) and exchanged with one more remote
    broadcast; result = (S(K1) - S(KLO)) / K2 on every core.
"""

import numpy as np

import concourse.bacc as bacc
import concourse.bass as bass
import concourse.bass_interp as bass_interp
import concourse.mybir as mybir
import concourse.tile as tile
from concourse import bass_isa
from concourse.bass_utils import run_bass_kernel_spmd

# The Tile scheduling pass simulates a single core, so semaphore
# increments that arrive from PEER cores (remote DMA broadcasts) never
# fire and the sim deadlocks. Pre-load those sems in the scheduling sim
# only -- the waits are real on hardware.
_REMOTE_SEMS: list = []
_orig_simulate = bass_interp.CoreSim.simulate


def _patched_simulate(self, *a, **kw):
    try:
        is_sched = self.is_scheduling_pass()
    except Exception:
        is_sched = False
    if _REMOTE_SEMS and is_sched:
        for sem in _REMOTE_SEMS:
            self.update_semaphore(
                bass.create_sync_update(sem, 1 << 20, skip_validation=True)
            )
    return _orig_simulate(self, *a, **kw)


bass_interp.CoreSim.simulate = _patched_simulate

N_CORES = 8
B, C = 4096, 32000
RPC = B // N_CORES  # rows per core
P = 128
NT = RPC // P  # row tiles per core
F = 4000  # free-dim chunk
NCH = C // F  # chunks per row tile

K1 = int(0.9 * B)  # 3686
K2 = int(0.7 * K1)  # 2580
KLO = K1 - K2  # 1106

f32 = mybir.dt.float32
i32 = mybir.dt.int32
AX = mybir.AxisListType.X
Alu = mybir.AluOpType
Act = mybir.ActivationFunctionType

GT = N_CORES * P  # values per gathered tile (1024)


def build():
    nc = bacc.Bacc(
        "TRN2", target_bir_lowering=False, debug=False, num_devices=N_CORES
    )
    x = nc.declare_dram_parameter("x", [RPC, C], f32, isOutput=False)
    tgt = nc.declare_dram_parameter("tgt", [RPC, 1], i32, isOutput=False)
    out = nc.declare_dram_parameter("out", [1, 1], f32, isOutput=True)
    # partition-major: loss_out[p, t] is the loss of local row t*128+p
    loss_out = nc.declare_dram_parameter("loss", [P, NT], f32, isOutput=True)

    # shared semaphores for the remote broadcasts (same numbering on all
    # cores -- SPMD). One per exchanged tile so arrival counts can't mix.
    rs_tile = [nc.alloc_semaphore(f"rs_t{t}") for t in range(NT)]
    rs_part = nc.alloc_semaphore("rs_part")
    ls_dummy = nc.alloc_semaphore("ls_dummy")
    _REMOTE_SEMS.clear()
    _REMOTE_SEMS.extend(rs_tile + [rs_part])

    def bcast(in_col, recv_tile, base_col, sem):
        """Send my [P,1-or-2] column to all 8 cores; receiver's slot j
        (at base_col + j widths) gets core (self XOR j)'s column.
        Returns (trigger, wait) so callers can pin consumers after the
        runtime arrival wait (the scheduling sim can't see remote
        increments, so without explicit edges it would hoist the wait)."""
        import bass_rust

        for j in range(N_CORES):
            rd = [None] * N_CORES
            rd[j] = (0, j)
            w = in_col.shape[-1]
            nc.gpsimd.remote_dma_broadcast(
                out_ap=recv_tile[:, base_col + j * w : base_col + (j + 1) * w],
                in_ap=in_col,
                remote_sem=sem,
                local_sem=ls_dummy,
                rdests=rd,
            )
        trig = nc.gpsimd.trigger_dma(count=None)
        wait = nc.gpsimd.wait_ge(sem, 16)
        bass_rust.add_dep_helper(
            wait.ins, trig.ins, sync=True, reason="arrival wait after trigger"
        )
        return trig, wait

    with tile.TileContext(nc) as tc:
        with (
            tc.tile_pool(name="chunk", bufs=8) as chunk_pool,
            tc.tile_pool(name="junk", bufs=3) as junk_pool,
            tc.tile_pool(name="stats", bufs=4) as stats,
            tc.tile_pool(name="persist", bufs=1) as persist,
            tc.tile_pool(name="dram", bufs=1, space="DRAM") as dram,
            tc.tile_pool(name="rjunk", bufs=1, space="PSUM") as rjunk_pool,
            tc.tile_pool(name="rsb", bufs=1) as rsb_pool,
        ):
            myvals = persist.tile([P, NT], f32)  # this core's losses
            s4 = persist.tile([P, NT], f32)  # per-tile exp-sums
            xt4 = persist.tile([P, NT], f32)  # gathered x[i, target[i]]
            expnx = persist.tile([P, NT], f32)  # exp(-x_t)
            gvals = persist.tile([P, NT], f32)  # s * exp(-x_t) = exp(loss)
            # remote-gather receive buffers: slot-major [P, 8] per tile
            recv = persist.tile([P, N_CORES * NT], f32)
            recv_p = persist.tile([P, 2 * N_CORES], f32)
            pr = persist.tile([P, 2], f32)
            # rank partials: [P, own_tile * NT + gathered_tile]
            rparts = persist.tile([P, NT * NT], f32)
            bounces = [
                dram.tile([P, N_CORES], f32, name=f"bounce{t}")
                for t in range(NT)
            ]
            la = persist.tile([1, GT * NT], f32)  # flattened gathered vals

            ones_t = persist.tile([1, P], f32)
            nc.vector.memset(ones_t[:], 1.0)
            # warm the ACT exp table while the first chunk DMA is in flight
            warm = persist.tile([P, 1], f32)
            nc.vector.memset(warm[:], 0.0)
            nc.scalar.activation(out=warm[:], in_=warm[:], func=Act.Exp)

            # dummy CC op: keeps has_collectives=True so the runtime does
            # full comm init (routing); finishes during the init barrier.
            gdum = dram.tile([1, 1], f32)
            pdum = dram.tile([N_CORES, 1], f32)
            nc.gpsimd.dma_start(out=gdum[:], in_=warm[0:1, 0:1])
            nc.gpsimd.collective_compute(
                "AllGather", Alu.bypass,
                replica_groups=[list(range(N_CORES))],
                ins=[gdum[:].opt()], outs=[pdum[:].opt()],
            )

            # ---- hoisted target gathers: x[i, target[i]] for ALL tiles ----
            tg = persist.tile([P, NT], i32)
            for ti in range(NT):
                nc.gpsimd.dma_start(
                    out=tg[:, ti : ti + 1], in_=tgt[ti * P : (ti + 1) * P, :]
                )
            ofs = persist.tile([P, NT], i32)
            for ti in range(NT):
                nc.gpsimd.iota(
                    ofs[:, ti : ti + 1],
                    pattern=[[0, 1]],
                    base=ti * P * C,
                    channel_multiplier=C,
                )
            nc.vector.tensor_add(out=ofs[:], in0=ofs[:], in1=tg[:])
            for ti in range(NT):
                nc.gpsimd.indirect_dma_start(
                    out=xt4[:, ti : ti + 1],
                    out_offset=None,
                    in_=x[:].rearrange("a (b one) -> (a b) one", one=1),
                    in_offset=bass.IndirectOffsetOnAxis(
                        ap=ofs[:, ti : ti + 1], axis=0
                    ),
                )
            nc.scalar.activation(
                out=expnx[:], in_=xt4[:], func=Act.Exp, scale=-1.0
            )

            # ---------------- phase 1: per-row NLL ----------------
            def do_tile(ti):
                acc = stats.tile([P, NCH + 3], f32, tag="acc")
                for ci in range(NCH):
                    lo, hi = F * ci, F * (ci + 1)
                    ch = chunk_pool.tile([P, F], f32, tag="chunk")
                    nc.sync.dma_start(
                        out=ch[:], in_=x[ti * P : (ti + 1) * P, lo:hi]
                    )
                    junk = junk_pool.tile([P, F], f32, tag="junk")
                    nc.scalar.activation(
                        out=junk[:],
                        in_=ch[:],
                        func=Act.Exp,
                        accum_out=acc[:, ci : ci + 1],
                    )
                nc.vector.reduce_sum(s4[:, ti : ti + 1], acc[:, :NCH], axis=AX)
                nc.vector.tensor_mul(
                    out=gvals[:, ti : ti + 1],
                    in0=s4[:, ti : ti + 1],
                    in1=expnx[:, ti : ti + 1],
                )

            lt = rjunk_pool.tile([P, GT * NT], f32, tag="lt_ps")
            rankjunk = rsb_pool.tile([P, GT], f32, tag="rank_junk")

            def exchange_tile(t):
                """Remote-broadcast gvals[:, t]; flatten the received slot
                tile to [1, GT]; ones-matmul it across partitions into
                PSUM for rank comparisons."""
                import bass_rust

                _, wait = bcast(
                    gvals[:, t : t + 1], recv, t * N_CORES, rs_tile[t]
                )
                cols = slice(t * N_CORES, (t + 1) * N_CORES)
                flat = nc.gpsimd.dma_start(
                    out=bounces[t][:], in_=recv[:, cols]
                )
                bass_rust.add_dep_helper(
                    flat.ins, wait.ins, sync=True,
                    reason="flatten after remote arrivals",
                )
                nc.gpsimd.dma_start(
                    out=la[:, t * GT : (t + 1) * GT],
                    in_=bounces[t][:]
                    .rearrange("a b -> (a b)")
                    .rearrange("(n one) -> one n", one=1),
                )
                for c in range(GT // 512):
                    o = t * GT + c * 512
                    nc.tensor.matmul(
                        out=lt[:, o : o + 512],
                        lhsT=ones_t[0:1, :],
                        rhs=la[0:1, o : o + 512],
                        start=True,
                        stop=True,
                    )

            def rank_pair(own_t, g_t):
                """rparts[:, own_t*NT+g_t] = count(gathered tile g_t < my
                gvals[:, own_t])."""
                nc.vector.tensor_scalar(
                    out=rankjunk[:],
                    in0=lt[:, g_t * GT : (g_t + 1) * GT],
                    scalar1=gvals[:, own_t : own_t + 1],
                    scalar2=0.0,
                    op0=Alu.is_lt,
                    op1=Alu.add,
                    accum_out=rparts[:, own_t * NT + g_t : own_t * NT + g_t + 1],
                )

            for ti in range(NT - 1):
                do_tile(ti)
                exchange_tile(ti)
                for own_t in range(ti + 1):
                    rank_pair(own_t, ti)
                    if own_t != ti:
                        rank_pair(ti, own_t)

            # losses for tiles 0..2: ACT slots the Ln (2 table switches)
            # into idle gaps between tile-3 chunk exps
            nc.scalar.activation(
                out=myvals[:, : NT - 1], in_=gvals[:, : NT - 1], func=Act.Ln
            )

            do_tile(NT - 1)

            # ---------------- tail ----------------
            exchange_tile(NT - 1)
            nc.scalar.activation(
                out=myvals[:, NT - 1 :], in_=gvals[:, NT - 1 :], func=Act.Ln
            )
            for g_t in range(NT - 1):
                rank_pair(NT - 1, g_t)
            for own_t in range(NT):
                rank_pair(own_t, NT - 1)

            ranks = persist.tile([P, NT], f32)
            nc.vector.reduce_sum(
                ranks[:],
                rparts[:].rearrange("p (t g) -> p t g", t=NT),
                axis=AX,
            )

            # band partials: S(k) = sum(v, rank<k), strict ranks
            red = stats.tile([P, 2], f32, tag="red")
            for j, k in enumerate((float(K1), float(KLO))):
                sel = stats.tile([P, NT], f32, tag="sel")
                nc.vector.tensor_scalar(
                    out=sel[:], in0=ranks[:], scalar1=k, scalar2=None,
                    op0=Alu.is_lt,
                )
                mv = stats.tile([P, NT], f32, tag="mv")
                nc.vector.tensor_mul(out=mv[:], in0=myvals[:], in1=sel[:])
                nc.vector.reduce_sum(red[:, j : j + 1], mv[:], axis=AX)
            nc.gpsimd.partition_all_reduce(
                pr[:], red[:], channels=P, reduce_op=bass_isa.ReduceOp.add
            )

            # exchange partials (every partition of pr holds the same [2])
            import bass_rust

            _, waitp = bcast(pr[:, 0:2], recv_p, 0, rs_part)
            sums = persist.tile([P, 2], f32)
            rsum = nc.vector.reduce_sum(
                sums[:],
                recv_p[:].rearrange("p (c s) -> p s c", s=2),
                axis=AX,
            )
            bass_rust.add_dep_helper(
                rsum.ins, waitp.ins, sync=True,
                reason="partials reduce after remote arrivals",
            )
            res = persist.tile([1, 1], f32)
            nc.vector.tensor_sub(
                out=res[:], in0=sums[0:1, 0:1], in1=sums[0:1, 1:2]
            )
            nc.vector.tensor_scalar(
                out=res[:],
                in0=res[:],
                scalar1=1.0 / K2,
                scalar2=None,
                op0=Alu.mult,
            )
            nc.gpsimd.dma_start(out=out[:], in_=res[:])
            # debug output, off the critical path
            nc.gpsimd.dma_start(out=loss_out[:], in_=myvals[:])

    nc.compile()
    return nc


_CACHE = {}


def _get_nc():
    if "nc" not in _CACHE:
        _CACHE["nc"] = build()
    return _CACHE["nc"]


def _in_maps(x, target):
    x = np.ascontiguousarray(np.asarray(x, dtype=np.float32))
    t = np.asarray(target).astype(np.int32).reshape(B, 1)
    return [
        {
            "x": x[c * RPC : (c + 1) * RPC],
            "tgt": np.ascontiguousarray(t[c * RPC : (c + 1) * RPC]),
        }
        for c in range(N_CORES)
    ]


def run(x, target, trace=False):
    nc = _get_nc()
    res = run_bass_kernel_spmd(
        nc, _in_maps(x, target), list(range(N_CORES)), trace=trace
    )
    val = np.asarray(res.results[0]["out"][0, 0], dtype=np.float32).reshape(())
    return val, res


def kernel(x, target):
    val, _ = run(x, target, trace=False)
    return val


# revision 29
# speedup vs baseline: 1.0065x; 1.0065x over previous
"""Trainium2 Bass kernel for OESM CrossEntropy (two-stage top-k band mean).

reference semantics:
    loss[i] = -log_softmax(x)[i, target[i]]            # [B]
    keep the k1 = int(0.9*B) smallest losses, then the k2 = int(0.7*k1)
    largest of those, return their mean.
Equivalently: mean of the losses with ascending rank in [k1-k2, k1).

Strategy (8 NeuronCores, SPMD):
  - rows sharded 512/core; per row: sum(exp(x)) via ScalarE Exp with
    accum_out, x[i, target[i]] via indirect DMA gather (hoisted to t=0),
    g = s * exp(-x_t) = exp(loss), a strictly monotone transform of loss.
  - the [B] g-vector is exchanged with direct SBUF->SBUF remote DMA
    broadcasts (XOR-relative dests), one per row tile, as soon as each
    tile's g column is ready -- tiles 0..2 land while tile 3 still
    streams, so their rank work fully overlaps the stream. The CC
    AllGather software path (~13-32us latency) is bypassed entirely;
    a dummy 4-byte CC AllGather at t=0 keeps comm init alive.
  - each core ranks its own 512 values against all 4096 (DVE
    tensor_scalar is_lt with accum_out over a PSUM ones-matmul
    broadcast of the gathered values). The remote gather order is a
    per-receiver XOR permutation, which is harmless: rank counts are
    order-invariant.
  - band sum with strict ranks (no tie correction; exact for distinct
    values): S(k) = sum(v, rank<k). Per-core partials are
    partition-reduced (gpsimd) and exchanged with one more remote
    broadcast; result = (S(K1) - S(KLO)) / K2 on every core.
"""

import numpy as np

import concourse.bacc as bacc
import concourse.bass as bass
import concourse.bass_interp as bass_interp
import concourse.mybir as mybir
import concourse.tile as tile
from concourse import bass_isa
from concourse.bass_utils import run_bass_kernel_spmd

# The Tile scheduling pass simulates a single core, so semaphore
# increments that arrive from PEER cores (remote DMA broadcasts) never
# fire and the sim deadlocks. Inject the arrivals as timed sim events at
# realistic times -- the scheduler then orders consumers of remote data
# where they belong (after the stream work live at that time) instead of
# hoisting them. The waits are real on hardware.
_REMOTE_ARRIVALS: list = []  # (sem_handle, arrival_ns, anchor_instruction)
_orig_simulate = bass_interp.CoreSim.simulate


def _patched_simulate(self, *a, **kw):
    try:
        is_sched = self.is_scheduling_pass()
    except Exception:
        is_sched = False
    if _REMOTE_ARRIVALS and is_sched:
        for sem, t_ns, inst in _REMOTE_ARRIVALS:
            upd = bass.create_sync_update(sem, 1 << 20, skip_validation=True)
            self.schedule_event(
                (lambda u=upd: self.update_semaphore(u)),
                int(t_ns),
                [],
                inst,
            )
    return _orig_simulate(self, *a, **kw)


bass_interp.CoreSim.simulate = _patched_simulate

N_CORES = 8
B, C = 4096, 32000
RPC = B // N_CORES  # rows per core
P = 128
NT = RPC // P  # row tiles per core
F = 4000  # free-dim chunk
NCH = C // F  # chunks per row tile

K1 = int(0.9 * B)  # 3686
K2 = int(0.7 * K1)  # 2580
KLO = K1 - K2  # 1106

f32 = mybir.dt.float32
i32 = mybir.dt.int32
AX = mybir.AxisListType.X
Alu = mybir.AluOpType
Act = mybir.ActivationFunctionType

GT = N_CORES * P  # values per gathered tile (1024)


def build():
    nc = bacc.Bacc(
        "TRN2", target_bir_lowering=False, debug=False, num_devices=N_CORES
    )
    x = nc.declare_dram_parameter("x", [RPC, C], f32, isOutput=False)
    tgt = nc.declare_dram_parameter("tgt", [RPC, 1], i32, isOutput=False)
    out = nc.declare_dram_parameter("out", [1, 1], f32, isOutput=True)
    # partition-major: loss_out[p, t] is the loss of local row t*128+p
    loss_out = nc.declare_dram_parameter("loss", [P, NT], f32, isOutput=True)

    # shared semaphores for the remote broadcasts (same numbering on all
    # cores -- SPMD). One per exchanged tile so arrival counts can't mix.
    rs_tile = [nc.alloc_semaphore(f"rs_t{t}") for t in range(NT)]
    rs_part = nc.alloc_semaphore("rs_part")
    ls_dummy = nc.alloc_semaphore("ls_dummy")
    _REMOTE_ARRIVALS.clear()

    def bcast(in_col, recv_tile, base_col, sem, arrival_ns):
        """Send my [P,1-or-2] column to all 8 cores; receiver's slot j
        (at base_col + j widths) gets core (self XOR j)'s column.
        Returns (trigger, wait) so callers can pin consumers after the
        runtime arrival wait. arrival_ns tells the scheduling sim when
        the remote increments land."""
        import bass_rust

        for j in range(N_CORES):
            rd = [None] * N_CORES
            rd[j] = (0, j)
            w = in_col.shape[-1]
            nc.gpsimd.remote_dma_broadcast(
                out_ap=recv_tile[:, base_col + j * w : base_col + (j + 1) * w],
                in_ap=in_col,
                remote_sem=sem,
                local_sem=ls_dummy,
                rdests=rd,
            )
        trig = nc.gpsimd.trigger_dma(count=None)
        wait = nc.gpsimd.wait_ge(sem, 16)
        bass_rust.add_dep_helper(
            wait.ins, trig.ins, sync=True, reason="arrival wait after trigger"
        )
        _REMOTE_ARRIVALS.append((sem, arrival_ns, wait.ins))
        return trig, wait

    with tile.TileContext(nc) as tc:
        with (
            tc.tile_pool(name="chunk", bufs=8) as chunk_pool,
            tc.tile_pool(name="junk", bufs=3) as junk_pool,
            tc.tile_pool(name="stats", bufs=4) as stats,
            tc.tile_pool(name="persist", bufs=1) as persist,
            tc.tile_pool(name="dram", bufs=1, space="DRAM") as dram,
            tc.tile_pool(name="rjunk", bufs=1, space="PSUM") as rjunk_pool,
            tc.tile_pool(name="rsb", bufs=1) as rsb_pool,
        ):
            myvals = persist.tile([P, NT], f32)  # this core's losses
            s4 = persist.tile([P, NT], f32)  # per-tile exp-sums
            xt4 = persist.tile([P, NT], f32)  # gathered x[i, target[i]]
            expnx = persist.tile([P, NT], f32)  # exp(-x_t)
            gvals = persist.tile([P, NT], f32)  # s * exp(-x_t) = exp(loss)
            # remote-gather receive buffers: slot-major [P, 8] per tile
            recv = persist.tile([P, N_CORES * NT], f32)
            recv_p = persist.tile([P, 2 * N_CORES], f32)
            pr = persist.tile([P, 2], f32)
            # rank partials: [P, own_tile * NT + gathered_tile]
            rparts = persist.tile([P, NT * NT], f32)
            bounces = [
                dram.tile([P, N_CORES], f32, name=f"bounce{t}")
                for t in range(NT)
            ]
            la = persist.tile([1, GT * NT], f32)  # flattened gathered vals

            ones_t = persist.tile([1, P], f32)
            nc.vector.memset(ones_t[:], 1.0)
            # warm the ACT exp table while the first chunk DMA is in flight
            warm = persist.tile([P, 1], f32)
            nc.vector.memset(warm[:], 0.0)
            nc.scalar.activation(out=warm[:], in_=warm[:], func=Act.Exp)

            # dummy CC op: keeps has_collectives=True so the runtime does
            # full comm init (routing); finishes during the init barrier.
            gdum = dram.tile([1, 1], f32)
            pdum = dram.tile([N_CORES, 1], f32)
            nc.gpsimd.dma_start(out=gdum[:], in_=warm[0:1, 0:1])
            nc.gpsimd.collective_compute(
                "AllGather", Alu.bypass,
                replica_groups=[list(range(N_CORES))],
                ins=[gdum[:].opt()], outs=[pdum[:].opt()],
            )

            # ---- hoisted target gathers: x[i, target[i]] for ALL tiles ----
            tg = persist.tile([P, NT], i32)
            for ti in range(NT):
                nc.gpsimd.dma_start(
                    out=tg[:, ti : ti + 1], in_=tgt[ti * P : (ti + 1) * P, :]
                )
            ofs = persist.tile([P, NT], i32)
            for ti in range(NT):
                nc.gpsimd.iota(
                    ofs[:, ti : ti + 1],
                    pattern=[[0, 1]],
                    base=ti * P * C,
                    channel_multiplier=C,
                )
            nc.vector.tensor_add(out=ofs[:], in0=ofs[:], in1=tg[:])
            for ti in range(NT):
                nc.gpsimd.indirect_dma_start(
                    out=xt4[:, ti : ti + 1],
                    out_offset=None,
                    in_=x[:].rearrange("a (b one) -> (a b) one", one=1),
                    in_offset=bass.IndirectOffsetOnAxis(
                        ap=ofs[:, ti : ti + 1], axis=0
                    ),
                )
            nc.scalar.activation(
                out=expnx[:], in_=xt4[:], func=Act.Exp, scale=-1.0
            )

            # ---------------- phase 1: per-row NLL ----------------
            def do_tile(ti):
                acc = stats.tile([P, NCH + 3], f32, tag="acc")
                for ci in range(NCH):
                    lo, hi = F * ci, F * (ci + 1)
                    ch = chunk_pool.tile([P, F], f32, tag="chunk")
                    nc.sync.dma_start(
                        out=ch[:], in_=x[ti * P : (ti + 1) * P, lo:hi]
                    )
                    junk = junk_pool.tile([P, F], f32, tag="junk")
                    nc.scalar.activation(
                        out=junk[:],
                        in_=ch[:],
                        func=Act.Exp,
                        accum_out=acc[:, ci : ci + 1],
                    )
                nc.vector.reduce_sum(s4[:, ti : ti + 1], acc[:, :NCH], axis=AX)
                nc.vector.tensor_mul(
                    out=gvals[:, ti : ti + 1],
                    in0=s4[:, ti : ti + 1],
                    in1=expnx[:, ti : ti + 1],
                )

            lt = rjunk_pool.tile([P, GT * NT], f32, tag="lt_ps")
            rankjunk = rsb_pool.tile([P, GT], f32, tag="rank_junk")

            def exchange_tile(t):
                """Remote-broadcast gvals[:, t]; flatten the received slot
                tile to [1, GT]; ones-matmul it across partitions into
                PSUM for rank comparisons."""
                import bass_rust

                # arrival estimate: tile t's exp-sum completes ~(58+52t)us
                # into the stream; preps+trigger+wire add ~9us
                _, wait = bcast(
                    gvals[:, t : t + 1], recv, t * N_CORES, rs_tile[t],
                    (67 + 52 * t) * 1000,
                )
                cols = slice(t * N_CORES, (t + 1) * N_CORES)
                flat = nc.gpsimd.dma_start(
                    out=bounces[t][:], in_=recv[:, cols]
                )
                bass_rust.add_dep_helper(
                    flat.ins, wait.ins, sync=True,
                    reason="flatten after remote arrivals",
                )
                nc.gpsimd.dma_start(
                    out=la[:, t * GT : (t + 1) * GT],
                    in_=bounces[t][:]
                    .rearrange("a b -> (a b)")
                    .rearrange("(n one) -> one n", one=1),
                )
                for c in range(GT // 512):
                    o = t * GT + c * 512
                    nc.tensor.matmul(
                        out=lt[:, o : o + 512],
                        lhsT=ones_t[0:1, :],
                        rhs=la[0:1, o : o + 512],
                        start=True,
                        stop=True,
                    )

            def rank_pair(own_t, g_t):
                """rparts[:, own_t*NT+g_t] = count(gathered tile g_t < my
                gvals[:, own_t])."""
                nc.vector.tensor_scalar(
                    out=rankjunk[:],
                    in0=lt[:, g_t * GT : (g_t + 1) * GT],
                    scalar1=gvals[:, own_t : own_t + 1],
                    scalar2=0.0,
                    op0=Alu.is_lt,
                    op1=Alu.add,
                    accum_out=rparts[:, own_t * NT + g_t : own_t * NT + g_t + 1],
                )

            for ti in range(NT - 1):
                do_tile(ti)
                exchange_tile(ti)
                for own_t in range(ti + 1):
                    rank_pair(own_t, ti)
                    if own_t != ti:
                        rank_pair(ti, own_t)

            # losses for tiles 0..2: ACT slots the Ln (2 table switches)
            # into idle gaps between tile-3 chunk exps
            nc.scalar.activation(
                out=myvals[:, : NT - 1], in_=gvals[:, : NT - 1], func=Act.Ln
            )

            do_tile(NT - 1)

            # ---------------- tail ----------------
            exchange_tile(NT - 1)
            nc.scalar.activation(
                out=myvals[:, NT - 1 :], in_=gvals[:, NT - 1 :], func=Act.Ln
            )
            for g_t in range(NT - 1):
                rank_pair(NT - 1, g_t)
            for own_t in range(NT):
                rank_pair(own_t, NT - 1)

            ranks = persist.tile([P, NT], f32)
            nc.vector.reduce_sum(
                ranks[:],
                rparts[:].rearrange("p (t g) -> p t g", t=NT),
                axis=AX,
            )

            # band partials: S(k) = sum(v, rank<k), strict ranks
            red = stats.tile([P, 2], f32, tag="red")
            for j, k in enumerate((float(K1), float(KLO))):
                sel = stats.tile([P, NT], f32, tag="sel")
                nc.vector.tensor_scalar(
                    out=sel[:], in0=ranks[:], scalar1=k, scalar2=None,
                    op0=Alu.is_lt,
                )
                mv = stats.tile([P, NT], f32, tag="mv")
                nc.vector.tensor_mul(out=mv[:], in0=myvals[:], in1=sel[:])
                nc.vector.reduce_sum(red[:, j : j + 1], mv[:], axis=AX)
            nc.gpsimd.partition_all_reduce(
                pr[:], red[:], channels=P, reduce_op=bass_isa.ReduceOp.add
            )

            # exchange partials (every partition of pr holds the same [2])
            import bass_rust

            _, waitp = bcast(pr[:, 0:2], recv_p, 0, rs_part, 248 * 1000)
            sums = persist.tile([P, 2], f32)
            rsum = nc.vector.reduce_sum(
                sums[:],
                recv_p[:].rearrange("p (c s) -> p s c", s=2),
                axis=AX,
            )
            bass_rust.add_dep_helper(
                rsum.ins, waitp.ins, sync=True,
                reason="partials reduce after remote arrivals",
            )
            res = persist.tile([1, 1], f32)
            nc.vector.tensor_sub(
                out=res[:], in0=sums[0:1, 0:1], in1=sums[0:1, 1:2]
            )
            nc.vector.tensor_scalar(
                out=res[:],
                in0=res[:],
                scalar1=1.0 / K2,
                scalar2=None,
                op0=Alu.mult,
            )
            nc.gpsimd.dma_start(out=out[:], in_=res[:])
            # debug output, off the critical path
            nc.gpsimd.dma_start(out=loss_out[:], in_=myvals[:])

    nc.compile()
    return nc


_CACHE = {}


def _get_nc():
    if "nc" not in _CACHE:
        _CACHE["nc"] = build()
    return _CACHE["nc"]


def _in_maps(x, target):
    x = np.ascontiguousarray(np.asarray(x, dtype=np.float32))
    t = np.asarray(target).astype(np.int32).reshape(B, 1)
    return [
        {
            "x": x[c * RPC : (c + 1) * RPC],
            "tgt": np.ascontiguousarray(t[c * RPC : (c + 1) * RPC]),
        }
        for c in range(N_CORES)
    ]


def run(x, target, trace=False):
    nc = _get_nc()
    res = run_bass_kernel_spmd(
        nc, _in_maps(x, target), list(range(N_CORES)), trace=trace
    )
    val = np.asarray(res.results[0]["out"][0, 0], dtype=np.float32).reshape(())
    return val, res


def kernel(x, target):
    val, _ = run(x, target, trace=False)
    return val
